# revision 47
# baseline (speedup 1.0000x reference)
"""Trainium2 Bass kernel for nn_EngramAttention (causal MHA block).

Computes: qkv = x @ Wqkv + bqkv; causal 16-head attention; out @ Wout + bout.
Shapes: x [2, 2048, 1024], Wqkv [1024, 3072], Wout [1024, 1024].

Sharding (8 NeuronCores, tensor-parallel by heads):
  - core c owns heads {2c, 2c+1} (128 feature columns of each of Q/K/V).
  - Every core reads all tokens (x fed pre-transposed, feature-major, bf16).
  - QKV projection, causal attention (scoresT layout: softmax across the
    partition axis via an appended ones-row in the PV matmul), producing the
    un-projected attention output feature-major [128, 4096] per core.
  - One AllToAll redistributes [head-features x token-chunk] blocks so each
    core ends with ALL 1024 features for its 512-token slice.
  - Each core runs the output projection for its token slice; host concatenates.

All matmuls run in bf16 (fp32 accumulation in PSUM).
"""

import os
import sys

for _p in ("/opt/trn_rl_repo", "/root/.axon_site/_ro/trn_rl_repo"):
    if os.path.isdir(_p) and _p not in sys.path:
        sys.path.insert(0, _p)

import ml_dtypes
import numpy as np

import concourse.bass as bass
import concourse.mybir as mybir
import concourse.tile as tile
from concourse.bass_utils import run_bass_kernel_spmd
from concourse.masks import make_identity
from concourse.vector_clock import ScopedClock

BF16 = mybir.dt.bfloat16
F32 = mybir.dt.float32
NPBF16 = ml_dtypes.bfloat16

NCORES = 8
D = 1024          # hidden
NTOK = 4096       # B*T
T = 2048
B = 2
FEAT = 128        # per-core head features (2 heads x 64)
TOKC = NTOK // NCORES  # 512 tokens per core in the output projection
SCALE = 0.125     # 1/sqrt(64)

# module-level handles for optional tracing by test harnesses
TRACE = False
TRACE_KWARGS = {}
LAST_RESULT = None


class _SplitDrainTileContext(tile.TileContext):
    """TileContext whose tail drain splits semaphore waits one-per-instruction.

    The walrus build in this container rejects >N sync waits on a single
    Drain ("Too many sync wait commands"), so emit a chain of drains, each
    carrying a single wait, instead of one drain carrying all of them.
    """

    def _drain_and_barrier(self, tick_clock, wait_clock):
        nc = self.nc
        drain_inst = nc.sync.drain()
        wait_clock.add_sem_waits(
            drain_inst.ins, ScopedClock({None: tick_clock.global_clock})
        )
        si = drain_inst.ins.sync_info
        if si is not None and si.on_wait and len(si.on_wait) > 1:
            waits = list(si.on_wait)
            drain_inst.ins.sync_info = mybir.SyncInfo(
                on_wait=waits[:1], on_update=list(si.on_update or [])
            )
            for w in waits[1:]:
                d2 = nc.sync.drain()
                si2 = d2.ins.sync_info
                upd = list(si2.on_update or []) if si2 is not None else []
                d2.ins.sync_info = mybir.SyncInfo(on_wait=[w], on_update=upd)

        nc.all_engine_barrier()
        assert self.sems is not None
        popped = nc._tile_sem_poison_stack.pop()
        assert popped is self._sem_poison
        nc.clear_and_free_semaphores(list(self.sems.allocated().values()))
        nc.all_engine_barrier()


def _split_excess_waits(nc, aux, max_waits=1):
    """Walrus in this container rejects instructions carrying more than a
    couple of semaphore waits ("Too many sync wait commands").  Move excess
    waits onto EventSemaphore carrier instructions inserted just before the
    offending instruction on the same engine (same-engine FIFO order makes
    this semantically identical).

    DMA instructions execute on the DMA-queue processors, asynchronously
    from the issuing engine's stream, so an engine-side carrier alone would
    NOT order them (CoreSim race detector confirms).  For those, the carrier
    chain additionally increments an auxiliary semaphore and the DMA itself
    waits on it — the DMA then carries exactly one wait."""
    n = 0
    aux_count = 0
    dma_ops = ("DMACopy", "DMATranspose", "TriggeredCopy")

    def _carrier(engine, wait_grp):
        nonlocal n
        ev = mybir.InstEventSemaphore(
            name=f"wsplit-{n}",
            engine=engine,
            ins=[],
            outs=[],
            sync_info=mybir.SyncInfo(on_wait=list(wait_grp), on_update=[]),
        )
        n += 1
        nc.register_instruction(ev, overwrite=True)
        return ev

    for fn in nc.m.functions:
        for blk in fn.blocks:
            out = []
            for ins in blk.instructions:
                si = ins.sync_info
                waits = list(si.on_wait) if (si is not None and si.on_wait) else []
                if len(waits) > max_waits:
                    if ins.opcode in dma_ops:
                        for w in waits:
                            out.append(_carrier(ins.engine, [w]))
                        bass.BassInstruction(out[-1]).then_inc(aux, 1)
                        aux_count += 1
                        ins.sync_info = mybir.SyncInfo(
                            on_wait=[], on_update=list(si.on_update or [])
                        )
                        bass.BassInstruction(ins).wait_op(
                            aux, aux_count, "sem-ge"
                        )
                    else:
                        extra, keep = waits[:-max_waits], waits[-max_waits:]
                        for i in range(0, len(extra), max_waits):
                            out.append(_carrier(ins.engine, extra[i : i + max_waits]))
                        ins.sync_info = mybir.SyncInfo(
                            on_wait=keep, on_update=list(si.on_update or [])
                        )
                out.append(ins)
            blk.instructions = out
    if aux_count:
        # sems persist across NEFF executions; reset so a re-run starts at 0
        nc.gpsimd.sem_clear(range(aux.num, aux.num + 1))
    return n


_STAGE = os.environ.get("KERNEL_STAGE", "full")


def _build_nc():
    nc = bass.Bass("TRN2", num_devices=NCORES)

    xT = nc.dram_tensor("xT", [D, NTOK], BF16, kind="ExternalInput")
    wq = nc.dram_tensor("wq", [D, FEAT], BF16, kind="ExternalInput")
    wk = nc.dram_tensor("wk", [D, FEAT], BF16, kind="ExternalInput")
    wv = nc.dram_tensor("wv", [D, FEAT], BF16, kind="ExternalInput")
    bq = nc.dram_tensor("bq", [FEAT, 1], F32, kind="ExternalInput")
    bk = nc.dram_tensor("bk", [FEAT, 1], F32, kind="ExternalInput")
    bv = nc.dram_tensor("bv", [FEAT, 1], F32, kind="ExternalInput")
    wout = nc.dram_tensor("wout", [D, D], BF16, kind="ExternalInput")
    boutb = nc.dram_tensor("boutb", [1, D], BF16, kind="ExternalInput")
    maskg = nc.dram_tensor("maskg", [128, 896], BF16, kind="ExternalInput")
    y = nc.dram_tensor("y", [TOKC, D], F32, kind="ExternalOutput")

    # auxiliary semaphore for the DMA-wait splitting pass; allocated (and
    # cleared) before the TileContext so Tile never recycles its ID
    aux_sem = nc.alloc_semaphore("wsplit_aux")
    nc.gpsimd.sem_clear(range(aux_sem.num, aux_sem.num + 1))

    with _SplitDrainTileContext(nc) as tc:
        with (
            tc.tile_pool(name="const", bufs=1) as cp,
            tc.tile_pool(name="work", bufs=2) as wp,
            tc.tile_pool(name="stage", bufs=2) as sp2,
            tc.tile_pool(name="psA", bufs=3, space="PSUM") as psA,
            tc.tile_pool(name="psB", bufs=2, space="PSUM") as psB,
            tc.tile_pool(name="dram", bufs=1, space="DRAM") as dp,
        ):
            # ---- persistent SBUF tensors ----
            xt_sb = cp.tile([128, 8 * NTOK], BF16, name="xt_sb")     # 64 KB/part
            wq_sb = cp.tile([128, 8 * FEAT], BF16, name="wq_sb")
            wk_sb = cp.tile([128, 8 * FEAT], BF16, name="wk_sb")
            wv_sb = cp.tile([128, 8 * FEAT], BF16, name="wv_sb")
            bq_sb = cp.tile([FEAT, 1], F32, name="bq_sb")
            bk_sb = cp.tile([FEAT, 1], F32, name="bk_sb")
            bv_sb = cp.tile([FEAT, 1], F32, name="bv_sb")
            bout_sb = cp.tile([1, D], BF16, name="bout_sb")
            mask_sb = cp.tile([128, 896], BF16, name="mask_sb")
            ident_sb = cp.tile([128, 128], BF16, name="ident_sb")
            ones1_sb = cp.tile([1, 128], BF16, name="ones1_sb")
            qT_sb = cp.tile([128, NTOK], BF16, name="qT_sb")
            kT_sb = cp.tile([128, NTOK], BF16, name="kT_sb")
            vtok_sb = cp.tile([128, 32 * 130], BF16, name="vtok_sb")
            attn_sb = cp.tile([128, NTOK], BF16, name="attn_sb")
            ag_sb = cp.tile([128, 8 * 512], BF16, name="ag_sb")
            # vT shares the big work slots; it dies after the transposes
            vT_sb = wp.tile([128, NTOK], BF16, tag="pt", name="vT_sb")

            # ---- input DMAs ----
            # xT goes first on the sync HWDGE ring (the QKV k-outer loop
            # streams right behind it); small weight/bias/mask loads go via
            # the gpsimd SWDGE ring so they don't delay xT.
            for kt in range(8):
                nc.sync.dma_start(
                    xt_sb[:, kt * NTOK : (kt + 1) * NTOK],
                    xT[kt * 128 : (kt + 1) * 128, :],
                )
            for w_sb, wdr in ((wq_sb, wq), (wk_sb, wk), (wv_sb, wv)):
                for kt in range(8):
                    nc.gpsimd.dma_start(
                        w_sb[:, kt * FEAT : (kt + 1) * FEAT],
                        wdr[kt * 128 : (kt + 1) * 128, :],
                    )
            nc.gpsimd.dma_start(bq_sb[:], bq[:])
            nc.gpsimd.dma_start(bk_sb[:], bk[:])
            nc.gpsimd.dma_start(bv_sb[:], bv[:])
            nc.gpsimd.dma_start(bout_sb[:], boutb[:])
            nc.gpsimd.dma_start(mask_sb[:], maskg[:])

            make_identity(nc, ident_sb[:])
            nc.vector.memset(ones1_sb[:], 1.0)
            vt_view = vtok_sb[:].rearrange("p (g c) -> p g c", c=130)
            nc.vector.memset(vt_view[:, :, 64], 1.0)
            nc.vector.memset(vt_view[:, :, 129], 1.0)
            if _STAGE == "qkv":
                nc.vector.memset(attn_sb[:], 0.0)

            # ---- QKV projection: dstT[f, tok] = W.T @ x.T (+ bias) ----
            # kt-outer over pairs of [128,1024] PSUM tiles so the PE starts
            # as soon as the first xT k-tile lands (overlaps the input DMA).
            for w_sb, b_sb, dst in (
                (wq_sb, bq_sb, qT_sb),
                (wk_sb, bk_sb, kT_sb),
                (wv_sb, bv_sb, vT_sb),
            ):
                for half in range(2):
                    pss = [
                        psA.tile([128, 1024], F32, tag="mm2", name=f"ps_qkv{t}")
                        for t in range(2)
                    ]
                    for kt in range(8):
                        for t in range(2):
                            base = (half * 2 + t) * 1024
                            for c in range(2):
                                nc.tensor.matmul(
                                    pss[t][:, c * 512 : (c + 1) * 512],
                                    w_sb[:, kt * FEAT : (kt + 1) * FEAT],
                                    xt_sb[
                                        :,
                                        kt * NTOK + base + c * 512 :
                                        kt * NTOK + base + (c + 1) * 512,
                                    ],
                                    start=(kt == 0),
                                    stop=(kt == 7),
                                )
                    for t in range(2):
                        base = (half * 2 + t) * 1024
                        nc.vector.tensor_scalar_add(
                            dst[:, base : base + 1024], pss[t][:], b_sb[:]
                        )

            # ---- v to token-major (PE transposes), with ones column ----
            for g in range(32):
                ps_t = psB.tile([128, 128], BF16, tag="pv")
                nc.tensor.transpose(
                    ps_t[:], vT_sb[:, g * 128 : (g + 1) * 128], ident_sb[:]
                )
                nc.vector.tensor_copy(
                    vtok_sb[:, g * 130 : g * 130 + 64], ps_t[:, 0:64]
                )
                nc.vector.tensor_copy(
                    vtok_sb[:, g * 130 + 65 : g * 130 + 129], ps_t[:, 64:128]
                )

            # ---- attention ----
            # Stages are (b, j); the two heads' score matmuls ALTERNATE
            # (row-groups 0:63 / 64:127) so each LDWEIGHTS overlaps the other
            # half's in-flight matmul — keeps the PE array duty near 100% so
            # the HAM activity monitor unthrottles the clock to 2.4 GHz.
            # Emission interleaves scores(s) with PV+norm(s-1) 1:1.
            stages = [(bb, j) for bb in range(2) for j in range(4)]
            if _STAGE == "qkv":
                stages = []
            pt_tiles = {}

            a2a_in = a2a_out = None
            if _STAGE == "full":
                a2a_in = dp.tile([8, 128, 512], BF16, name="a2a_in")
                a2a_out = dp.tile([8, 128, 512], BF16, name="a2a_out")

            def scores_ops(s):
                bb, j = s
                cb = bb * T
                nk = 4 * (j + 1)
                # pt layout: [h block (nk*512)] x 2
                pt = wp.tile(
                    [128, 2 * 16 * 512], BF16, tag="pt", name=f"pt_{bb}_{j}"
                )
                pt_tiles[s] = pt
                ops = []
                for kp in range(nk // 2):
                    def op(kp=kp, nk=nk, pt=pt, cb=cb, j=j):
                        ps2 = [
                            psA.tile([128, 1024], F32, tag="mm2", name=f"ps_s{h}")
                            for h in range(2)
                        ]
                        for c in range(2):
                            kk = 2 * kp + c
                            for h in range(2):
                                pb = 64 * h
                                nc.tensor.matmul(
                                    ps2[h][:, c * 512 : (c + 1) * 512],
                                    kT_sb[
                                        pb : pb + 64,
                                        cb + kk * 128 : cb + (kk + 1) * 128,
                                    ],
                                    qT_sb[
                                        pb : pb + 64,
                                        cb + j * 512 : cb + (j + 1) * 512,
                                    ],
                                    start=True,
                                    stop=True,
                                )
                        for h in range(2):
                            base = h * nk * 512
                            nc.scalar.activation(
                                pt[:, base + 2 * kp * 512 : base + (2 * kp + 2) * 512],
                                ps2[h][:],
                                mybir.ActivationFunctionType.Exp,
                                scale=SCALE,
                            )
                            for c in range(2):
                                kk = 2 * kp + c
                                if kk >= 4 * j:
                                    i = kk - 4 * j
                                    nc.vector.tensor_tensor(
                                        pt[:, base + kk * 512 : base + (kk + 1) * 512],
                                        pt[:, base + kk * 512 : base + (kk + 1) * 512],
                                        mask_sb[:, 384 - 128 * i : 896 - 128 * i],
                                        mybir.AluOpType.mult,
                                    )
                    ops.append(op)
                return ops

            def pv_ops(s):
                bb, j = s
                cb = bb * T
                nk = 4 * (j + 1)
                pt = pt_tiles.pop(s)
                ops = []
                for h in range(2):
                    pb = 64 * h
                    base = h * nk * 512
                    ps_box = {}
                    for kk in range(nk):
                        def op(kk=kk, nk=nk, pt=pt, base=base, h=h, bb=bb,
                               ps_box=ps_box):
                            if kk == 0:
                                ps_box["o"] = psB.tile(
                                    [65, 512], F32, tag="pv", name="ps_o"
                                )
                            g = bb * 16 + kk
                            nc.tensor.matmul(
                                ps_box["o"][:],
                                vtok_sb[:, g * 130 + 65 * h : g * 130 + 65 * h + 65],
                                pt[:, base + kk * 512 : base + (kk + 1) * 512],
                                start=(kk == 0),
                                stop=(kk == nk - 1),
                            )
                        ops.append(op)

                    def norm(j=j, pb=pb, cb=cb, h=h, bb=bb, ps_box=ps_box):
                        ps_o = ps_box["o"]
                        srow = sp2.tile([1, 512], F32, tag="srow")
                        srowb = sp2.tile([1, 512], BF16, tag="srowb")
                        nc.vector.tensor_copy(srow[:], ps_o[64:65, :])
                        nc.vector.reciprocal(srow[:], srow[:])
                        nc.scalar.copy(srowb[:], srow[:])
                        sl_p = slice(pb, pb + 64)
                        sl_c = slice(cb + j * 512, cb + (j + 1) * 512)
                        nc.vector.tensor_copy(attn_sb[sl_p, sl_c], ps_o[0:64, :])
                        ps_bc = psB.tile([64, 512], F32, tag="pv", name="ps_bc")
                        nc.tensor.matmul(
                            ps_bc[:], ones1_sb[0:1, 0:64], srowb[:],
                            start=True, stop=True,
                        )
                        nc.vector.tensor_tensor(
                            attn_sb[sl_p, sl_c],
                            attn_sb[sl_p, sl_c],
                            ps_bc[:],
                            mybir.AluOpType.mult,
                        )
                        if h == 1 and _STAGE == "full":
                            jj = bb * 4 + j
                            nc.sync.dma_start(
                                a2a_in[jj],
                                attn_sb[:, jj * 512 : (jj + 1) * 512],
                            )
                    ops.append(norm)
                return ops

            def emit_interleaved(a_ops, b_ops):
                if not a_ops:
                    for op in b_ops:
                        op()
                    return
                nb = len(b_ops)
                done = 0
                for i, op in enumerate(a_ops):
                    op()
                    want = (i + 1) * nb // len(a_ops)
                    while done < want:
                        b_ops[done]()
                        done += 1

            prev = None
            for s in stages:
                a = scores_ops(s)
                b = pv_ops(prev) if prev is not None else []
                emit_interleaved(a, b)
                prev = s
            if prev is not None:
                for op in pv_ops(prev):
                    op()
            if _STAGE == "full":
                nc.gpsimd.collective_compute(
                    "AllToAll",
                    mybir.AluOpType.bypass,
                    replica_groups=[list(range(NCORES))],
                    ins=[a2a_in[:]],
                    outs=[a2a_out[:]],
                )
                for kt in range(8):
                    nc.sync.dma_start(
                        ag_sb[:, kt * 512 : (kt + 1) * 512], a2a_out[kt]
                    )
            else:
                nc.vector.tensor_copy(ag_sb[:], attn_sb[:])

            # wout loads into a freed big slot during late attention
            wout_sb = wp.tile([128, 8 * D], BF16, tag="pt", name="wout_sb")
            for kt in range(8):
                nc.sync.dma_start(
                    wout_sb[:, kt * D : (kt + 1) * D],
                    wout[kt * 128 : (kt + 1) * 128, :],
                )

            # ---- output projection for this core's 512-token slice ----
            for m in range(4):
                y_sb = wp.tile([128, D], F32, tag="pt")
                ps_y = psA.tile([128, 1024], F32, tag="mm2")
                for n2 in range(2):
                    nc.tensor.matmul(
                        ps_y[:, n2 * 512 : (n2 + 1) * 512],
                        ones1_sb[0:1, 0:128],
                        bout_sb[:, n2 * 512 : (n2 + 1) * 512],
                        start=True,
                        stop=False,
                    )
                    for kt in range(8):
                        nc.tensor.matmul(
                            ps_y[:, n2 * 512 : (n2 + 1) * 512],
                            ag_sb[:, kt * 512 + m * 128 : kt * 512 + (m + 1) * 128],
                            wout_sb[:, kt * D + n2 * 512 : kt * D + (n2 + 1) * 512],
                            start=False,
                            stop=(kt == 7),
                        )
                nc.vector.tensor_copy(y_sb[:], ps_y[:])
                nc.sync.dma_start(y[m * 128 : (m + 1) * 128, :], y_sb[:])

    _split_excess_waits(nc, aux_sem)
    return nc


_NC_CACHE = None


def _get_nc():
    global _NC_CACHE
    if _NC_CACHE is None:
        _NC_CACHE = _build_nc()
    return _NC_CACHE


def kernel(x, Wqkv, bqkv, Wout, bout):
    global LAST_RESULT
    x = np.asarray(x, dtype=np.float32)
    Wqkv = np.asarray(Wqkv, dtype=np.float32)
    bqkv = np.asarray(bqkv, dtype=np.float32)
    Wout = np.asarray(Wout, dtype=np.float32)
    bout = np.asarray(bout, dtype=np.float32)

    Bx, Tx, Dx = x.shape
    assert (Bx, Tx, Dx) == (B, T, D)

    xT = np.ascontiguousarray(x.reshape(NTOK, D).T).astype(NPBF16)
    wq_full = Wqkv[:, 0:D]
    wk_full = Wqkv[:, D : 2 * D]
    wv_full = Wqkv[:, 2 * D : 3 * D]
    bq_full = bqkv[0:D]
    bk_full = bqkv[D : 2 * D]
    bv_full = bqkv[2 * D : 3 * D]

    wout_b = np.ascontiguousarray(Wout).astype(NPBF16)
    boutb = np.ascontiguousarray(bout.reshape(1, D)).astype(NPBF16)
    maskg = (
        np.arange(896)[None, :] >= (np.arange(128)[:, None] + 384)
    ).astype(NPBF16)

    in_maps = []
    for c in range(NCORES):
        sl = slice(FEAT * c, FEAT * (c + 1))
        in_maps.append(
            {
                "xT": xT,
                "wq": np.ascontiguousarray(wq_full[:, sl]).astype(NPBF16),
                "wk": np.ascontiguousarray(wk_full[:, sl]).astype(NPBF16),
                "wv": np.ascontiguousarray(wv_full[:, sl]).astype(NPBF16),
                "bq": np.ascontiguousarray(bq_full[sl].reshape(FEAT, 1)),
                "bk": np.ascontiguousarray(bk_full[sl].reshape(FEAT, 1)),
                "bv": np.ascontiguousarray(bv_full[sl].reshape(FEAT, 1)),
                "wout": wout_b,
                "boutb": boutb,
                "maskg": maskg,
            }
        )

    nc = _get_nc()
    res = run_bass_kernel_spmd(
        nc,
        in_maps,
        core_ids=list(range(NCORES)),
        trace=TRACE,
        **TRACE_KWARGS,
    )
    LAST_RESULT = res
    ys = [res.results[c]["y"] for c in range(NCORES)]
    out = np.concatenate(ys, axis=0).reshape(B, T, D).astype(np.float32)
    return out


# revision 48
# speedup vs baseline: 1.0944x; 1.0944x over previous
"""Trainium2 Bass kernel for nn_EngramAttention (causal MHA block).

Computes: qkv = x @ Wqkv + bqkv; causal 16-head attention; out @ Wout + bout.
Shapes: x [2, 2048, 1024], Wqkv [1024, 3072], Wout [1024, 1024].

Sharding (8 NeuronCores, tensor-parallel by heads):
  - core c owns heads {2c, 2c+1} (128 feature columns of each of Q/K/V).
  - Every core reads all tokens (x fed pre-transposed, feature-major, bf16).
  - QKV projection, causal attention (scoresT layout: softmax across the
    partition axis via an appended ones-row in the PV matmul), producing the
    un-projected attention output feature-major [128, 4096] per core.
  - One AllToAll redistributes [head-features x token-chunk] blocks so each
    core ends with ALL 1024 features for its 512-token slice.
  - Each core runs the output projection for its token slice; host concatenates.

All matmuls run in bf16 (fp32 accumulation in PSUM).
"""

import os
import sys

for _p in ("/opt/trn_rl_repo", "/root/.axon_site/_ro/trn_rl_repo"):
    if os.path.isdir(_p) and _p not in sys.path:
        sys.path.insert(0, _p)

import ml_dtypes
import numpy as np

import concourse.bass as bass
import concourse.mybir as mybir
import concourse.tile as tile
from concourse.bass_utils import run_bass_kernel_spmd
from concourse.masks import make_identity
from concourse.vector_clock import ScopedClock

BF16 = mybir.dt.bfloat16
F32 = mybir.dt.float32
NPBF16 = ml_dtypes.bfloat16

NCORES = 8
D = 1024          # hidden
NTOK = 4096       # B*T
T = 2048
B = 2
FEAT = 128        # per-core head features (2 heads x 64)
TOKC = NTOK // NCORES  # 512 tokens per core in the output projection
SCALE = 0.125     # 1/sqrt(64)

# module-level handles for optional tracing by test harnesses
TRACE = False
TRACE_KWARGS = {}
LAST_RESULT = None


class _SplitDrainTileContext(tile.TileContext):
    """TileContext whose tail drain splits semaphore waits one-per-instruction.

    The walrus build in this container rejects >N sync waits on a single
    Drain ("Too many sync wait commands"), so emit a chain of drains, each
    carrying a single wait, instead of one drain carrying all of them.
    """

    def _drain_and_barrier(self, tick_clock, wait_clock):
        nc = self.nc
        drain_inst = nc.sync.drain()
        wait_clock.add_sem_waits(
            drain_inst.ins, ScopedClock({None: tick_clock.global_clock})
        )
        si = drain_inst.ins.sync_info
        if si is not None and si.on_wait and len(si.on_wait) > 1:
            waits = list(si.on_wait)
            drain_inst.ins.sync_info = mybir.SyncInfo(
                on_wait=waits[:1], on_update=list(si.on_update or [])
            )
            for w in waits[1:]:
                d2 = nc.sync.drain()
                si2 = d2.ins.sync_info
                upd = list(si2.on_update or []) if si2 is not None else []
                d2.ins.sync_info = mybir.SyncInfo(on_wait=[w], on_update=upd)

        nc.all_engine_barrier()
        assert self.sems is not None
        popped = nc._tile_sem_poison_stack.pop()
        assert popped is self._sem_poison
        nc.clear_and_free_semaphores(list(self.sems.allocated().values()))
        nc.all_engine_barrier()


def _split_excess_waits(nc, aux, max_waits=1):
    """Walrus in this container rejects instructions carrying more than a
    couple of semaphore waits ("Too many sync wait commands").  Move excess
    waits onto EventSemaphore carrier instructions inserted just before the
    offending instruction on the same engine (same-engine FIFO order makes
    this semantically identical).

    DMA instructions execute on the DMA-queue processors, asynchronously
    from the issuing engine's stream, so an engine-side carrier alone would
    NOT order them (CoreSim race detector confirms).  For those, the carrier
    chain additionally increments an auxiliary semaphore and the DMA itself
    waits on it — the DMA then carries exactly one wait."""
    n = 0
    aux_count = 0
    dma_ops = ("DMACopy", "DMATranspose", "TriggeredCopy")

    def _carrier(engine, wait_grp):
        nonlocal n
        ev = mybir.InstEventSemaphore(
            name=f"wsplit-{n}",
            engine=engine,
            ins=[],
            outs=[],
            sync_info=mybir.SyncInfo(on_wait=list(wait_grp), on_update=[]),
        )
        n += 1
        nc.register_instruction(ev, overwrite=True)
        return ev

    for fn in nc.m.functions:
        for blk in fn.blocks:
            out = []
            for ins in blk.instructions:
                si = ins.sync_info
                waits = list(si.on_wait) if (si is not None and si.on_wait) else []
                if len(waits) > max_waits:
                    if ins.opcode in dma_ops:
                        for w in waits:
                            out.append(_carrier(ins.engine, [w]))
                        bass.BassInstruction(out[-1]).then_inc(aux, 1)
                        aux_count += 1
                        ins.sync_info = mybir.SyncInfo(
                            on_wait=[], on_update=list(si.on_update or [])
                        )
                        bass.BassInstruction(ins).wait_op(
                            aux, aux_count, "sem-ge"
                        )
                    else:
                        extra, keep = waits[:-max_waits], waits[-max_waits:]
                        for i in range(0, len(extra), max_waits):
                            out.append(_carrier(ins.engine, extra[i : i + max_waits]))
                        ins.sync_info = mybir.SyncInfo(
                            on_wait=keep, on_update=list(si.on_update or [])
                        )
                out.append(ins)
            blk.instructions = out
    if aux_count:
        # sems persist across NEFF executions; reset so a re-run starts at 0
        nc.gpsimd.sem_clear(range(aux.num, aux.num + 1))
    return n


_STAGE = os.environ.get("KERNEL_STAGE", "full")


def _build_nc():
    nc = bass.Bass("TRN2", num_devices=NCORES)

    xT = nc.dram_tensor("xT", [D, NTOK], BF16, kind="ExternalInput")
    wq = nc.dram_tensor("wq", [D, FEAT], BF16, kind="ExternalInput")
    wk = nc.dram_tensor("wk", [D, FEAT], BF16, kind="ExternalInput")
    wv = nc.dram_tensor("wv", [D, FEAT], BF16, kind="ExternalInput")
    bq = nc.dram_tensor("bq", [FEAT, 1], F32, kind="ExternalInput")
    bk = nc.dram_tensor("bk", [FEAT, 1], F32, kind="ExternalInput")
    bv = nc.dram_tensor("bv", [FEAT, 1], F32, kind="ExternalInput")
    wout = nc.dram_tensor("wout", [D, D], BF16, kind="ExternalInput")
    boutb = nc.dram_tensor("boutb", [1, D], BF16, kind="ExternalInput")
    maskg = nc.dram_tensor("maskg", [128, 896], BF16, kind="ExternalInput")
    y = nc.dram_tensor("y", [TOKC, D], F32, kind="ExternalOutput")

    # auxiliary semaphore for the DMA-wait splitting pass; allocated (and
    # cleared) before the TileContext so Tile never recycles its ID
    aux_sem = nc.alloc_semaphore("wsplit_aux")
    nc.gpsimd.sem_clear(range(aux_sem.num, aux_sem.num + 1))

    with _SplitDrainTileContext(nc) as tc:
        with (
            tc.tile_pool(name="const", bufs=1) as cp,
            tc.tile_pool(name="work", bufs=2) as wp,
            tc.tile_pool(name="stage", bufs=2) as sp2,
            tc.tile_pool(name="psA", bufs=3, space="PSUM") as psA,
            tc.tile_pool(name="psB", bufs=2, space="PSUM") as psB,
            tc.tile_pool(name="dram", bufs=1, space="DRAM") as dp,
        ):
            # ---- persistent SBUF tensors ----
            xt_sb = cp.tile([128, 8 * NTOK], BF16, name="xt_sb")     # 64 KB/part
            wq_sb = cp.tile([128, 8 * FEAT], BF16, name="wq_sb")
            wk_sb = cp.tile([128, 8 * FEAT], BF16, name="wk_sb")
            wv_sb = cp.tile([128, 8 * FEAT], BF16, name="wv_sb")
            bq_sb = cp.tile([FEAT, 1], F32, name="bq_sb")
            bk_sb = cp.tile([FEAT, 1], F32, name="bk_sb")
            bv_sb = cp.tile([FEAT, 1], F32, name="bv_sb")
            bout_sb = cp.tile([1, D], BF16, name="bout_sb")
            mask_sb = cp.tile([128, 896], BF16, name="mask_sb")
            ident_sb = cp.tile([128, 128], BF16, name="ident_sb")
            ones1_sb = cp.tile([1, 128], BF16, name="ones1_sb")
            qT_sb = cp.tile([128, NTOK], BF16, name="qT_sb")
            kT_sb = cp.tile([128, NTOK], BF16, name="kT_sb")
            vtok_sb = cp.tile([128, 32 * 130], BF16, name="vtok_sb")
            attn_sb = cp.tile([128, NTOK], BF16, name="attn_sb")
            ag_sb = cp.tile([128, 8 * 512], BF16, name="ag_sb")
            # vT shares the big work slots; it dies after the transposes
            vT_sb = wp.tile([128, NTOK], BF16, tag="pt", name="vT_sb")

            # ---- input DMAs ----
            # xT goes first on the sync HWDGE ring (the QKV k-outer loop
            # streams right behind it); small weight/bias/mask loads go via
            # the gpsimd SWDGE ring so they don't delay xT.
            for kt in range(8):
                nc.sync.dma_start(
                    xt_sb[:, kt * NTOK : (kt + 1) * NTOK],
                    xT[kt * 128 : (kt + 1) * 128, :],
                )
            for w_sb, wdr in ((wq_sb, wq), (wk_sb, wk), (wv_sb, wv)):
                for kt in range(8):
                    nc.gpsimd.dma_start(
                        w_sb[:, kt * FEAT : (kt + 1) * FEAT],
                        wdr[kt * 128 : (kt + 1) * 128, :],
                    )
            nc.gpsimd.dma_start(bq_sb[:], bq[:])
            nc.gpsimd.dma_start(bk_sb[:], bk[:])
            nc.gpsimd.dma_start(bv_sb[:], bv[:])
            nc.gpsimd.dma_start(bout_sb[:], boutb[:])
            nc.gpsimd.dma_start(mask_sb[:], maskg[:])

            make_identity(nc, ident_sb[:])
            nc.vector.memset(ones1_sb[:], 1.0)
            vt_view = vtok_sb[:].rearrange("p (g c) -> p g c", c=130)
            nc.vector.memset(vt_view[:, :, 64], 1.0)
            nc.vector.memset(vt_view[:, :, 129], 1.0)
            if _STAGE == "qkv":
                nc.vector.memset(attn_sb[:], 0.0)

            # ---- QKV projection: dstT[f, tok] = W.T @ x.T (+ bias) ----
            # kt-outer over pairs of [128,1024] PSUM tiles so the PE starts
            # as soon as the first xT k-tile lands (overlaps the input DMA).
            for w_sb, b_sb, dst in (
                (wq_sb, bq_sb, qT_sb),
                (wk_sb, bk_sb, kT_sb),
                (wv_sb, bv_sb, vT_sb),
            ):
                for half in range(2):
                    pss = [
                        psA.tile([128, 1024], F32, tag="mm2", name=f"ps_qkv{t}")
                        for t in range(2)
                    ]
                    for kt in range(8):
                        for t in range(2):
                            base = (half * 2 + t) * 1024
                            for c in range(2):
                                nc.tensor.matmul(
                                    pss[t][:, c * 512 : (c + 1) * 512],
                                    w_sb[:, kt * FEAT : (kt + 1) * FEAT],
                                    xt_sb[
                                        :,
                                        kt * NTOK + base + c * 512 :
                                        kt * NTOK + base + (c + 1) * 512,
                                    ],
                                    start=(kt == 0),
                                    stop=(kt == 7),
                                )
                    for t in range(2):
                        base = (half * 2 + t) * 1024
                        nc.vector.tensor_scalar_add(
                            dst[:, base : base + 1024], pss[t][:], b_sb[:]
                        )

            # ---- v to token-major (PE transposes), with ones column ----
            for g in range(32):
                ps_t = psB.tile([128, 128], BF16, tag="pv")
                nc.tensor.transpose(
                    ps_t[:], vT_sb[:, g * 128 : (g + 1) * 128], ident_sb[:]
                )
                nc.vector.tensor_copy(
                    vtok_sb[:, g * 130 : g * 130 + 64], ps_t[:, 0:64]
                )
                nc.vector.tensor_copy(
                    vtok_sb[:, g * 130 + 65 : g * 130 + 129], ps_t[:, 64:128]
                )

            # ---- attention ----
            # Stages merge two j-chunks: s = (h, b, jp), jp in {0,1} covering
            # j in {2jp, 2jp+1}.  h is OUTER so the head-0 AllToAll can run on
            # the TOPSP while head-1 attention computes.  Emission interleaves
            # scores(s) with PV+norm(s-1) 1:1 so the PE stream never stalls
            # long on the ACT-paced exp.
            stages = [
                (h, bb, jp) for h in range(2) for bb in range(2) for jp in range(2)
            ]
            if _STAGE == "qkv":
                stages = []
            pt_tiles = {}
            # flat pt column offset (in 512-blocks) for (j, kk)
            _joff = {0: 0, 1: 4, 2: 0, 3: 12}

            a2a1_in = a2a1_out = a2a2_in = a2a2_out = None
            if _STAGE == "full":
                a2a1_in = dp.tile([8, 64, 512], BF16, name="a2a1_in")
                a2a1_out = dp.tile([8, 64, 512], BF16, name="a2a1_out")
                a2a2_in = dp.tile([8, 64, 512], BF16, name="a2a2_in")
                a2a2_out = dp.tile([8, 64, 512], BF16, name="a2a2_out")

            def scores_ops(s):
                h, bb, jp = s
                pb, cb = 64 * h, bb * T
                pt = wp.tile(
                    [128, 28 * 512], BF16, tag="pt", name=f"pt_{h}_{bb}_{jp}"
                )
                pt_tiles[s] = pt
                ops = []
                for j in (2 * jp, 2 * jp + 1):
                    nk = 4 * (j + 1)
                    for kp in range(nk // 2):
                        def op(j=j, kp=kp, nk=nk, pt=pt, pb=pb, cb=cb):
                            off = _joff[j]
                            ps2 = psA.tile([128, 1024], F32, tag="mm2")
                            for c in range(2):
                                kk = 2 * kp + c
                                nc.tensor.matmul(
                                    ps2[:, c * 512 : (c + 1) * 512],
                                    kT_sb[
                                        pb : pb + 64,
                                        cb + kk * 128 : cb + (kk + 1) * 128,
                                    ],
                                    qT_sb[
                                        pb : pb + 64,
                                        cb + j * 512 : cb + (j + 1) * 512,
                                    ],
                                    start=True,
                                    stop=True,
                                )
                            nc.scalar.activation(
                                pt[
                                    :,
                                    (off + 2 * kp) * 512 : (off + 2 * kp + 2) * 512,
                                ],
                                ps2[:],
                                mybir.ActivationFunctionType.Exp,
                                scale=SCALE,
                            )
                            for c in range(2):
                                kk = 2 * kp + c
                                if kk >= 4 * j:
                                    i = kk - 4 * j
                                    nc.vector.tensor_tensor(
                                        pt[:, (off + kk) * 512 : (off + kk + 1) * 512],
                                        pt[:, (off + kk) * 512 : (off + kk + 1) * 512],
                                        mask_sb[:, 384 - 128 * i : 896 - 128 * i],
                                        mybir.AluOpType.mult,
                                    )
                        ops.append(op)
                return ops

            def pv_ops(s):
                h, bb, jp = s
                pb, cb = 64 * h, bb * T
                pt = pt_tiles.pop(s)
                ops = []
                for j in (2 * jp, 2 * jp + 1):
                    nk = 4 * (j + 1)
                    ps_box = {}
                    for kk in range(nk):
                        def op(j=j, kk=kk, nk=nk, pt=pt, pb=pb, cb=cb, bb=bb,
                               h=h, ps_box=ps_box):
                            if kk == 0:
                                ps_box["o"] = psB.tile(
                                    [65, 512], F32, tag="pv", name="ps_o"
                                )
                            off = _joff[j]
                            g = bb * 16 + kk
                            nc.tensor.matmul(
                                ps_box["o"][:],
                                vtok_sb[:, g * 130 + 65 * h : g * 130 + 65 * h + 65],
                                pt[:, (off + kk) * 512 : (off + kk + 1) * 512],
                                start=(kk == 0),
                                stop=(kk == nk - 1),
                            )
                        ops.append(op)

                    def norm(j=j, pb=pb, cb=cb, h=h, bb=bb, ps_box=ps_box):
                        ps_o = ps_box["o"]
                        srow = sp2.tile([1, 512], F32, tag="srow")
                        srowb = sp2.tile([1, 512], BF16, tag="srowb")
                        nc.vector.tensor_copy(srow[:], ps_o[64:65, :])
                        nc.vector.reciprocal(srow[:], srow[:])
                        nc.scalar.copy(srowb[:], srow[:])
                        ps_bc = psB.tile([64, 512], F32, tag="pv", name="ps_bc")
                        nc.tensor.matmul(
                            ps_bc[:], ones1_sb[0:1, 0:64], srowb[:],
                            start=True, stop=True,
                        )
                        sl_p = slice(pb, pb + 64)
                        sl_c = slice(cb + j * 512, cb + (j + 1) * 512)
                        nc.vector.tensor_copy(attn_sb[sl_p, sl_c], ps_o[0:64, :])
                        nc.vector.tensor_tensor(
                            attn_sb[sl_p, sl_c],
                            attn_sb[sl_p, sl_c],
                            ps_bc[:],
                            mybir.AluOpType.mult,
                        )
                        if _STAGE == "full":
                            jj = bb * 4 + j
                            a_in = a2a1_in if h == 0 else a2a2_in
                            nc.sync.dma_start(
                                a_in[jj],
                                attn_sb[pb : pb + 64, jj * 512 : (jj + 1) * 512],
                            )
                    ops.append(norm)
                return ops

            def emit_interleaved(a_ops, b_ops):
                if not a_ops:
                    for op in b_ops:
                        op()
                    return
                nb = len(b_ops)
                done = 0
                for i, op in enumerate(a_ops):
                    op()
                    want = (i + 1) * nb // len(a_ops)
                    while done < want:
                        b_ops[done]()
                        done += 1

            def emit_collective(half):
                if _STAGE != "full":
                    return
                a_in, a_out = (
                    (a2a1_in, a2a1_out) if half == 0 else (a2a2_in, a2a2_out)
                )
                nc.gpsimd.collective_compute(
                    "AllToAll",
                    mybir.AluOpType.bypass,
                    replica_groups=[list(range(NCORES))],
                    ins=[a_in[:]],
                    outs=[a_out[:]],
                )
                for kt in range(8):
                    nc.sync.dma_start(
                        ag_sb[64 * half : 64 * half + 64, kt * 512 : (kt + 1) * 512],
                        a_out[kt],
                    )

            prev = None
            for s in stages:
                a = scores_ops(s)
                b = pv_ops(prev) if prev is not None else []
                emit_interleaved(a, b)
                if prev is not None and prev == (0, 1, 1):
                    emit_collective(0)  # head-0 A2A overlaps head-1 compute
                prev = s
            if prev is not None:
                for op in pv_ops(prev):
                    op()
                emit_collective(1)
            if _STAGE != "full":
                nc.vector.tensor_copy(ag_sb[:], attn_sb[:])

            # wout loads into a freed big slot during late attention
            wout_sb = wp.tile([128, 8 * D], BF16, tag="pt", name="wout_sb")
            for kt in range(8):
                nc.sync.dma_start(
                    wout_sb[:, kt * D : (kt + 1) * D],
                    wout[kt * 128 : (kt + 1) * 128, :],
                )

            # ---- output projection for this core's 512-token slice ----
            for m in range(4):
                y_sb = wp.tile([128, D], F32, tag="pt")
                ps_y = psA.tile([128, 1024], F32, tag="mm2")
                for n2 in range(2):
                    nc.tensor.matmul(
                        ps_y[:, n2 * 512 : (n2 + 1) * 512],
                        ones1_sb[0:1, 0:128],
                        bout_sb[:, n2 * 512 : (n2 + 1) * 512],
                        start=True,
                        stop=False,
                    )
                    for kt in range(8):
                        nc.tensor.matmul(
                            ps_y[:, n2 * 512 : (n2 + 1) * 512],
                            ag_sb[:, kt * 512 + m * 128 : kt * 512 + (m + 1) * 128],
                            wout_sb[:, kt * D + n2 * 512 : kt * D + (n2 + 1) * 512],
                            start=False,
                            stop=(kt == 7),
                        )
                nc.vector.tensor_copy(y_sb[:], ps_y[:])
                nc.sync.dma_start(y[m * 128 : (m + 1) * 128, :], y_sb[:])

    _split_excess_waits(nc, aux_sem)
    return nc


_NC_CACHE = None


def _get_nc():
    global _NC_CACHE
    if _NC_CACHE is None:
        _NC_CACHE = _build_nc()
    return _NC_CACHE


def kernel(x, Wqkv, bqkv, Wout, bout):
    global LAST_RESULT
    x = np.asarray(x, dtype=np.float32)
    Wqkv = np.asarray(Wqkv, dtype=np.float32)
    bqkv = np.asarray(bqkv, dtype=np.float32)
    Wout = np.asarray(Wout, dtype=np.float32)
    bout = np.asarray(bout, dtype=np.float32)

    Bx, Tx, Dx = x.shape
    assert (Bx, Tx, Dx) == (B, T, D)

    xT = np.ascontiguousarray(x.reshape(NTOK, D).T).astype(NPBF16)
    wq_full = Wqkv[:, 0:D]
    wk_full = Wqkv[:, D : 2 * D]
    wv_full = Wqkv[:, 2 * D : 3 * D]
    bq_full = bqkv[0:D]
    bk_full = bqkv[D : 2 * D]
    bv_full = bqkv[2 * D : 3 * D]

    wout_b = np.ascontiguousarray(Wout).astype(NPBF16)
    boutb = np.ascontiguousarray(bout.reshape(1, D)).astype(NPBF16)
    maskg = (
        np.arange(896)[None, :] >= (np.arange(128)[:, None] + 384)
    ).astype(NPBF16)

    in_maps = []
    for c in range(NCORES):
        sl = slice(FEAT * c, FEAT * (c + 1))
        in_maps.append(
            {
                "xT": xT,
                "wq": np.ascontiguousarray(wq_full[:, sl]).astype(NPBF16),
                "wk": np.ascontiguousarray(wk_full[:, sl]).astype(NPBF16),
                "wv": np.ascontiguousarray(wv_full[:, sl]).astype(NPBF16),
                "bq": np.ascontiguousarray(bq_full[sl].reshape(FEAT, 1)),
                "bk": np.ascontiguousarray(bk_full[sl].reshape(FEAT, 1)),
                "bv": np.ascontiguousarray(bv_full[sl].reshape(FEAT, 1)),
                "wout": wout_b,
                "boutb": boutb,
                "maskg": maskg,
            }
        )

    nc = _get_nc()
    res = run_bass_kernel_spmd(
        nc,
        in_maps,
        core_ids=list(range(NCORES)),
        trace=TRACE,
        **TRACE_KWARGS,
    )
    LAST_RESULT = res
    ys = [res.results[c]["y"] for c in range(NCORES)]
    out = np.concatenate(ys, axis=0).reshape(B, T, D).astype(np.float32)
    return out


# revision 50
# speedup vs baseline: 1.0977x; 1.0030x over previous
"""Trainium2 Bass kernel for nn_EngramAttention (causal MHA block).

Computes: qkv = x @ Wqkv + bqkv; causal 16-head attention; out @ Wout + bout.
Shapes: x [2, 2048, 1024], Wqkv [1024, 3072], Wout [1024, 1024].

Sharding (8 NeuronCores, tensor-parallel by heads):
  - core c owns heads {2c, 2c+1} (128 feature columns of each of Q/K/V).
  - Every core reads all tokens (x fed pre-transposed, feature-major, bf16).
  - QKV projection, causal attention (scoresT layout: softmax across the
    partition axis via an appended ones-row in the PV matmul), producing the
    un-projected attention output feature-major [128, 4096] per core.
  - One AllToAll redistributes [head-features x token-chunk] blocks so each
    core ends with ALL 1024 features for its 512-token slice.
  - Each core runs the output projection for its token slice; host concatenates.

All matmuls run in bf16 (fp32 accumulation in PSUM).
"""

import os
import sys

for _p in ("/opt/trn_rl_repo", "/root/.axon_site/_ro/trn_rl_repo"):
    if os.path.isdir(_p) and _p not in sys.path:
        sys.path.insert(0, _p)

import ml_dtypes
import numpy as np

import concourse.bass as bass
import concourse.mybir as mybir
import concourse.tile as tile
from concourse.bass_utils import run_bass_kernel_spmd
from concourse.masks import make_identity
from concourse.vector_clock import ScopedClock

BF16 = mybir.dt.bfloat16
F32 = mybir.dt.float32
NPBF16 = ml_dtypes.bfloat16

NCORES = 8
D = 1024          # hidden
NTOK = 4096       # B*T
T = 2048
B = 2
FEAT = 128        # per-core head features (2 heads x 64)
TOKC = NTOK // NCORES  # 512 tokens per core in the output projection
SCALE = 0.125     # 1/sqrt(64)

# module-level handles for optional tracing by test harnesses
TRACE = False
TRACE_KWARGS = {}
LAST_RESULT = None


class _SplitDrainTileContext(tile.TileContext):
    """TileContext whose tail drain splits semaphore waits one-per-instruction.

    The walrus build in this container rejects >N sync waits on a single
    Drain ("Too many sync wait commands"), so emit a chain of drains, each
    carrying a single wait, instead of one drain carrying all of them.
    """

    def _drain_and_barrier(self, tick_clock, wait_clock):
        nc = self.nc
        drain_inst = nc.sync.drain()
        wait_clock.add_sem_waits(
            drain_inst.ins, ScopedClock({None: tick_clock.global_clock})
        )
        si = drain_inst.ins.sync_info
        if si is not None and si.on_wait and len(si.on_wait) > 1:
            waits = list(si.on_wait)
            drain_inst.ins.sync_info = mybir.SyncInfo(
                on_wait=waits[:1], on_update=list(si.on_update or [])
            )
            for w in waits[1:]:
                d2 = nc.sync.drain()
                si2 = d2.ins.sync_info
                upd = list(si2.on_update or []) if si2 is not None else []
                d2.ins.sync_info = mybir.SyncInfo(on_wait=[w], on_update=upd)

        nc.all_engine_barrier()
        assert self.sems is not None
        popped = nc._tile_sem_poison_stack.pop()
        assert popped is self._sem_poison
        nc.clear_and_free_semaphores(list(self.sems.allocated().values()))
        nc.all_engine_barrier()


def _split_excess_waits(nc, aux, max_waits=1):
    """Walrus in this container rejects instructions carrying more than a
    couple of semaphore waits ("Too many sync wait commands").  Move excess
    waits onto EventSemaphore carrier instructions inserted just before the
    offending instruction on the same engine (same-engine FIFO order makes
    this semantically identical).

    DMA instructions execute on the DMA-queue processors, asynchronously
    from the issuing engine's stream, so an engine-side carrier alone would
    NOT order them (CoreSim race detector confirms).  For those, the carrier
    chain additionally increments an auxiliary semaphore and the DMA itself
    waits on it — the DMA then carries exactly one wait."""
    n = 0
    aux_count = 0
    dma_ops = ("DMACopy", "DMATranspose", "TriggeredCopy")

    def _carrier(engine, wait_grp):
        nonlocal n
        ev = mybir.InstEventSemaphore(
            name=f"wsplit-{n}",
            engine=engine,
            ins=[],
            outs=[],
            sync_info=mybir.SyncInfo(on_wait=list(wait_grp), on_update=[]),
        )
        n += 1
        nc.register_instruction(ev, overwrite=True)
        return ev

    for fn in nc.m.functions:
        for blk in fn.blocks:
            out = []
            for ins in blk.instructions:
                si = ins.sync_info
                waits = list(si.on_wait) if (si is not None and si.on_wait) else []
                if len(waits) > max_waits:
                    if ins.opcode in dma_ops:
                        for w in waits:
                            out.append(_carrier(ins.engine, [w]))
                        bass.BassInstruction(out[-1]).then_inc(aux, 1)
                        aux_count += 1
                        ins.sync_info = mybir.SyncInfo(
                            on_wait=[], on_update=list(si.on_update or [])
                        )
                        bass.BassInstruction(ins).wait_op(
                            aux, aux_count, "sem-ge"
                        )
                    else:
                        extra, keep = waits[:-max_waits], waits[-max_waits:]
                        for i in range(0, len(extra), max_waits):
                            out.append(_carrier(ins.engine, extra[i : i + max_waits]))
                        ins.sync_info = mybir.SyncInfo(
                            on_wait=keep, on_update=list(si.on_update or [])
                        )
                out.append(ins)
            blk.instructions = out
    if aux_count:
        # sems persist across NEFF executions; reset so a re-run starts at 0
        nc.gpsimd.sem_clear(range(aux.num, aux.num + 1))
    return n


_STAGE = os.environ.get("KERNEL_STAGE", "full")


def _build_nc():
    nc = bass.Bass("TRN2", num_devices=NCORES)

    xT = nc.dram_tensor("xT", [D, NTOK], BF16, kind="ExternalInput")
    wq = nc.dram_tensor("wq", [D, FEAT], BF16, kind="ExternalInput")
    wk = nc.dram_tensor("wk", [D, FEAT], BF16, kind="ExternalInput")
    wv = nc.dram_tensor("wv", [D, FEAT], BF16, kind="ExternalInput")
    bq = nc.dram_tensor("bq", [FEAT, 1], F32, kind="ExternalInput")
    bk = nc.dram_tensor("bk", [FEAT, 1], F32, kind="ExternalInput")
    bv = nc.dram_tensor("bv", [FEAT, 1], F32, kind="ExternalInput")
    wout = nc.dram_tensor("wout", [D, D], BF16, kind="ExternalInput")
    boutb = nc.dram_tensor("boutb", [1, D], BF16, kind="ExternalInput")
    maskg = nc.dram_tensor("maskg", [128, 896], BF16, kind="ExternalInput")
    y = nc.dram_tensor("y", [TOKC, D], F32, kind="ExternalOutput")

    # auxiliary semaphore for the DMA-wait splitting pass; allocated (and
    # cleared) before the TileContext so Tile never recycles its ID
    aux_sem = nc.alloc_semaphore("wsplit_aux")
    nc.gpsimd.sem_clear(range(aux_sem.num, aux_sem.num + 1))

    with _SplitDrainTileContext(nc) as tc:
        with (
            tc.tile_pool(name="const", bufs=1) as cp,
            tc.tile_pool(name="work", bufs=2) as wp,
            tc.tile_pool(name="stage", bufs=2) as sp2,
            tc.tile_pool(name="psA", bufs=3, space="PSUM") as psA,
            tc.tile_pool(name="psB", bufs=2, space="PSUM") as psB,
            tc.tile_pool(name="dram", bufs=1, space="DRAM") as dp,
        ):
            # ---- persistent SBUF tensors ----
            xt_sb = cp.tile([128, 8 * NTOK], BF16, name="xt_sb")     # 64 KB/part
            wq_sb = cp.tile([128, 8 * FEAT], BF16, name="wq_sb")
            wk_sb = cp.tile([128, 8 * FEAT], BF16, name="wk_sb")
            wv_sb = cp.tile([128, 8 * FEAT], BF16, name="wv_sb")
            bq_sb = cp.tile([FEAT, 1], F32, name="bq_sb")
            bk_sb = cp.tile([FEAT, 1], F32, name="bk_sb")
            bv_sb = cp.tile([FEAT, 1], F32, name="bv_sb")
            bout_sb = cp.tile([1, D], BF16, name="bout_sb")
            mask_sb = cp.tile([128, 896], BF16, name="mask_sb")
            ident_sb = cp.tile([128, 128], BF16, name="ident_sb")
            ones1_sb = cp.tile([1, 128], BF16, name="ones1_sb")
            qT_sb = cp.tile([128, NTOK], BF16, name="qT_sb")
            kT_sb = cp.tile([128, NTOK], BF16, name="kT_sb")
            vtok_sb = cp.tile([128, 32 * 130], BF16, name="vtok_sb")
            attn_sb = cp.tile([128, NTOK], BF16, name="attn_sb")
            ag_sb = cp.tile([128, 8 * 512], BF16, name="ag_sb")
            # vT shares the big work slots; it dies after the transposes
            vT_sb = wp.tile([128, NTOK], BF16, tag="pt", name="vT_sb")

            # ---- input DMAs ----
            # xT goes first on the sync HWDGE ring (the QKV k-outer loop
            # streams right behind it); small weight/bias/mask loads go via
            # the gpsimd SWDGE ring so they don't delay xT.
            for kt in range(8):
                nc.sync.dma_start(
                    xt_sb[:, kt * NTOK : (kt + 1) * NTOK],
                    xT[kt * 128 : (kt + 1) * 128, :],
                )
            for w_sb, wdr in ((wq_sb, wq), (wk_sb, wk), (wv_sb, wv)):
                for kt in range(8):
                    nc.gpsimd.dma_start(
                        w_sb[:, kt * FEAT : (kt + 1) * FEAT],
                        wdr[kt * 128 : (kt + 1) * 128, :],
                    )
            nc.gpsimd.dma_start(bq_sb[:], bq[:])
            nc.gpsimd.dma_start(bk_sb[:], bk[:])
            nc.gpsimd.dma_start(bv_sb[:], bv[:])
            nc.gpsimd.dma_start(bout_sb[:], boutb[:])
            nc.gpsimd.dma_start(mask_sb[:], maskg[:])

            make_identity(nc, ident_sb[:])
            nc.vector.memset(ones1_sb[:], 1.0)
            vt_view = vtok_sb[:].rearrange("p (g c) -> p g c", c=130)
            nc.vector.memset(vt_view[:, :, 64], 1.0)
            nc.vector.memset(vt_view[:, :, 129], 1.0)
            if _STAGE == "qkv":
                nc.vector.memset(attn_sb[:], 0.0)

            # ---- QKV projection: dstT[f, tok] = W.T @ x.T (+ bias) ----
            # kt-outer over pairs of [128,1024] PSUM tiles so the PE starts
            # as soon as the first xT k-tile lands (overlaps the input DMA).
            for w_sb, b_sb, dst in (
                (wq_sb, bq_sb, qT_sb),
                (wk_sb, bk_sb, kT_sb),
                (wv_sb, bv_sb, vT_sb),
            ):
                for half in range(2):
                    pss = [
                        psA.tile([128, 1024], F32, tag="mm2", name=f"ps_qkv{t}")
                        for t in range(2)
                    ]
                    for kt in range(8):
                        for t in range(2):
                            base = (half * 2 + t) * 1024
                            for c in range(2):
                                nc.tensor.matmul(
                                    pss[t][:, c * 512 : (c + 1) * 512],
                                    w_sb[:, kt * FEAT : (kt + 1) * FEAT],
                                    xt_sb[
                                        :,
                                        kt * NTOK + base + c * 512 :
                                        kt * NTOK + base + (c + 1) * 512,
                                    ],
                                    start=(kt == 0),
                                    stop=(kt == 7),
                                )
                    for t in range(2):
                        base = (half * 2 + t) * 1024
                        nc.vector.tensor_scalar_add(
                            dst[:, base : base + 1024], pss[t][:], b_sb[:]
                        )

            # ---- v to token-major (PE transposes), with ones column ----
            for g in range(32):
                ps_t = psB.tile([128, 128], BF16, tag="pv")
                nc.tensor.transpose(
                    ps_t[:], vT_sb[:, g * 128 : (g + 1) * 128], ident_sb[:]
                )
                nc.vector.tensor_copy(
                    vtok_sb[:, g * 130 : g * 130 + 64], ps_t[:, 0:64]
                )
                nc.vector.tensor_copy(
                    vtok_sb[:, g * 130 + 65 : g * 130 + 129], ps_t[:, 64:128]
                )

            # ---- attention ----
            # Stages merge two j-chunks: s = (h, b, jp), jp in {0,1} covering
            # j in {2jp, 2jp+1}.  h is OUTER so the head-0 AllToAll can run on
            # the TOPSP while head-1 attention computes.  Emission interleaves
            # scores(s) with PV+norm(s-1) 1:1 so the PE stream never stalls
            # long on the ACT-paced exp.
            stages = [
                (h, bb, jp) for h in range(2) for bb in range(2) for jp in range(2)
            ]
            if _STAGE == "qkv":
                stages = []
            pt_tiles = {}
            # flat pt column offset (in 512-blocks) for (j, kk)
            _joff = {0: 0, 1: 4, 2: 0, 3: 12}

            a2a1_in = a2a1_out = a2a2_in = a2a2_out = None
            if _STAGE == "full":
                a2a1_in = dp.tile([8, 64, 512], BF16, name="a2a1_in")
                a2a1_out = dp.tile([8, 64, 512], BF16, name="a2a1_out")
                a2a2_in = dp.tile([8, 64, 512], BF16, name="a2a2_in")
                a2a2_out = dp.tile([8, 64, 512], BF16, name="a2a2_out")

            def scores_ops(s):
                h, bb, jp = s
                pb, cb = 64 * h, bb * T
                pt = wp.tile(
                    [128, 28 * 512], BF16, tag="pt", name=f"pt_{h}_{bb}_{jp}"
                )
                pt_tiles[s] = pt
                ops = []
                for j in (2 * jp, 2 * jp + 1):
                    nk = 4 * (j + 1)
                    for kp in range(nk // 2):
                        def op(j=j, kp=kp, nk=nk, pt=pt, pb=pb, cb=cb):
                            off = _joff[j]
                            ps2 = psA.tile([128, 1024], F32, tag="mm2")
                            for c in range(2):
                                kk = 2 * kp + c
                                nc.tensor.matmul(
                                    ps2[:, c * 512 : (c + 1) * 512],
                                    kT_sb[
                                        pb : pb + 64,
                                        cb + kk * 128 : cb + (kk + 1) * 128,
                                    ],
                                    qT_sb[
                                        pb : pb + 64,
                                        cb + j * 512 : cb + (j + 1) * 512,
                                    ],
                                    start=True,
                                    stop=True,
                                )
                            nc.scalar.activation(
                                pt[
                                    :,
                                    (off + 2 * kp) * 512 : (off + 2 * kp + 2) * 512,
                                ],
                                ps2[:],
                                mybir.ActivationFunctionType.Exp,
                                scale=SCALE,
                            )
                            for c in range(2):
                                kk = 2 * kp + c
                                if kk >= 4 * j:
                                    i = kk - 4 * j
                                    nc.vector.tensor_tensor(
                                        pt[:, (off + kk) * 512 : (off + kk + 1) * 512],
                                        pt[:, (off + kk) * 512 : (off + kk + 1) * 512],
                                        mask_sb[:, 384 - 128 * i : 896 - 128 * i],
                                        mybir.AluOpType.mult,
                                    )
                        ops.append(op)
                return ops

            def pv_ops(s):
                h, bb, jp = s
                pb, cb = 64 * h, bb * T
                pt = pt_tiles.pop(s)
                ops = []
                for j in (2 * jp, 2 * jp + 1):
                    nk = 4 * (j + 1)
                    ps_box = {}
                    for kk in range(nk):
                        def op(j=j, kk=kk, nk=nk, pt=pt, pb=pb, cb=cb, bb=bb,
                               h=h, ps_box=ps_box):
                            if kk == 0:
                                ps_box["o"] = psB.tile(
                                    [65, 512], F32, tag="pv", name="ps_o"
                                )
                            off = _joff[j]
                            g = bb * 16 + kk
                            nc.tensor.matmul(
                                ps_box["o"][:],
                                vtok_sb[:, g * 130 + 65 * h : g * 130 + 65 * h + 65],
                                pt[:, (off + kk) * 512 : (off + kk + 1) * 512],
                                start=(kk == 0),
                                stop=(kk == nk - 1),
                            )
                        ops.append(op)

                    def norm(j=j, pb=pb, cb=cb, h=h, bb=bb, ps_box=ps_box):
                        ps_o = ps_box["o"]
                        srow = sp2.tile([1, 512], F32, tag="srow")
                        srowb = sp2.tile([1, 512], BF16, tag="srowb")
                        nc.vector.tensor_copy(srow[:], ps_o[64:65, :])
                        nc.vector.reciprocal(srow[:], srow[:])
                        nc.scalar.copy(srowb[:], srow[:])
                        ps_bc = psB.tile([64, 512], F32, tag="pv", name="ps_bc")
                        nc.tensor.matmul(
                            ps_bc[:], ones1_sb[0:1, 0:64], srowb[:],
                            start=True, stop=True,
                        )
                        sl_p = slice(pb, pb + 64)
                        sl_c = slice(cb + j * 512, cb + (j + 1) * 512)
                        nc.vector.tensor_copy(attn_sb[sl_p, sl_c], ps_o[0:64, :])
                        nc.vector.tensor_tensor(
                            attn_sb[sl_p, sl_c],
                            attn_sb[sl_p, sl_c],
                            ps_bc[:],
                            mybir.AluOpType.mult,
                        )
                        if _STAGE == "full":
                            jj = bb * 4 + j
                            a_in = a2a1_in if h == 0 else a2a2_in
                            nc.sync.dma_start(
                                a_in[jj],
                                attn_sb[pb : pb + 64, jj * 512 : (jj + 1) * 512],
                            )
                    ops.append(norm)
                return ops

            def emit_interleaved(a_ops, b_ops):
                if not a_ops:
                    for op in b_ops:
                        op()
                    return
                nb = len(b_ops)
                done = 0
                for i, op in enumerate(a_ops):
                    op()
                    want = (i + 1) * nb // len(a_ops)
                    while done < want:
                        b_ops[done]()
                        done += 1

            def emit_collective(half):
                if _STAGE != "full":
                    return
                a_in, a_out = (
                    (a2a1_in, a2a1_out) if half == 0 else (a2a2_in, a2a2_out)
                )
                nc.gpsimd.collective_compute(
                    "AllToAll",
                    mybir.AluOpType.bypass,
                    replica_groups=[list(range(NCORES))],
                    ins=[a_in[:]],
                    outs=[a_out[:]],
                )
                for kt in range(8):
                    nc.sync.dma_start(
                        ag_sb[64 * half : 64 * half + 64, kt * 512 : (kt + 1) * 512],
                        a_out[kt],
                    )

            prev = None
            for s in stages:
                a = scores_ops(s)
                b = pv_ops(prev) if prev is not None else []
                emit_interleaved(a, b)
                if prev is not None and prev == (0, 1, 1):
                    emit_collective(0)  # head-0 A2A overlaps head-1 compute
                prev = s
            if prev is not None:
                for op in pv_ops(prev):
                    op()
                emit_collective(1)
            if _STAGE != "full":
                nc.vector.tensor_copy(ag_sb[:], attn_sb[:])

            # wout loads into a freed big slot during late attention
            wout_sb = wp.tile([128, 8 * D], BF16, tag="pt", name="wout_sb")
            for kt in range(8):
                nc.sync.dma_start(
                    wout_sb[:, kt * D : (kt + 1) * D],
                    wout[kt * 128 : (kt + 1) * 128, :],
                )

            # ---- output projection for this core's 512-token slice ----
            for m in range(4):
                y_sb = wp.tile([128, D], F32, tag="pt")
                ps_y = psA.tile([128, 1024], F32, tag="mm2")
                for n2 in range(2):
                    nc.tensor.matmul(
                        ps_y[:, n2 * 512 : (n2 + 1) * 512],
                        ones1_sb[0:1, 0:128],
                        bout_sb[:, n2 * 512 : (n2 + 1) * 512],
                        start=True,
                        stop=False,
                    )
                    for kt in range(8):
                        nc.tensor.matmul(
                            ps_y[:, n2 * 512 : (n2 + 1) * 512],
                            ag_sb[:, kt * 512 + m * 128 : kt * 512 + (m + 1) * 128],
                            wout_sb[:, kt * D + n2 * 512 : kt * D + (n2 + 1) * 512],
                            start=False,
                            stop=(kt == 7),
                        )
                nc.vector.tensor_copy(y_sb[:], ps_y[:])
                nc.sync.dma_start(y[m * 128 : (m + 1) * 128, :], y_sb[:])

    _split_excess_waits(nc, aux_sem)
    return nc


_NC_CACHE = None


def _get_nc():
    global _NC_CACHE
    if _NC_CACHE is None:
        _NC_CACHE = _build_nc()
    return _NC_CACHE


def kernel(x, Wqkv, bqkv, Wout, bout):
    global LAST_RESULT
    x = np.asarray(x, dtype=np.float32)
    Wqkv = np.asarray(Wqkv, dtype=np.float32)
    bqkv = np.asarray(bqkv, dtype=np.float32)
    Wout = np.asarray(Wout, dtype=np.float32)
    bout = np.asarray(bout, dtype=np.float32)

    Bx, Tx, Dx = x.shape
    assert (Bx, Tx, Dx) == (B, T, D)

    xT = np.ascontiguousarray(x.reshape(NTOK, D).T).astype(NPBF16)
    wq_full = Wqkv[:, 0:D]
    wk_full = Wqkv[:, D : 2 * D]
    wv_full = Wqkv[:, 2 * D : 3 * D]
    bq_full = bqkv[0:D]
    bk_full = bqkv[D : 2 * D]
    bv_full = bqkv[2 * D : 3 * D]

    wout_b = np.ascontiguousarray(Wout).astype(NPBF16)
    boutb = np.ascontiguousarray(bout.reshape(1, D)).astype(NPBF16)
    maskg = (
        np.arange(896)[None, :] >= (np.arange(128)[:, None] + 384)
    ).astype(NPBF16)

    in_maps = []
    for c in range(NCORES):
        sl = slice(FEAT * c, FEAT * (c + 1))
        in_maps.append(
            {
                "xT": xT,
                "wq": np.ascontiguousarray(wq_full[:, sl]).astype(NPBF16),
                "wk": np.ascontiguousarray(wk_full[:, sl]).astype(NPBF16),
                "wv": np.ascontiguousarray(wv_full[:, sl]).astype(NPBF16),
                "bq": np.ascontiguousarray(bq_full[sl].reshape(FEAT, 1)),
                "bk": np.ascontiguousarray(bk_full[sl].reshape(FEAT, 1)),
                "bv": np.ascontiguousarray(bv_full[sl].reshape(FEAT, 1)),
                "wout": wout_b,
                "boutb": boutb,
                "maskg": maskg,
            }
        )

    nc = _get_nc()
    res = run_bass_kernel_spmd(
        nc,
        in_maps,
        core_ids=list(range(NCORES)),
        trace=TRACE,
        **TRACE_KWARGS,
    )
    LAST_RESULT = res
    ys = [res.results[c]["y"] for c in range(NCORES)]
    out = np.concatenate(ys, axis=0).reshape(B, T, D).astype(np.float32)
    return out
